# revision 22
# baseline (speedup 1.0000x reference)
"""Trainium2 Bass kernel for the differentiable-Kalman-filter loss.

Math: the reference runs a T=100000-step linear recurrence
  x_{i+1} = M x_i + K obs[i-1],  eps_i = obs[i] - C x_{i+1},  M = A - K C
and accumulates yvar = sum outer(eps_i) + decaying P-terms, loss = slogdet(yvar/T).
rho(M) ~ 0.963, so the recurrence has ~400-step memory: eps becomes a truncated
causal convolution of obs.  Each core computes eps for a 12160-row slab via a
two-level blocked conv (B=16 within-block taps as one 512x512 triangular matmul,
block-boundary states from J=8 block-level taps), then accumulates the Gram
E^T E on-chip.  The first W=2720 rows + the tiny P-series are computed exactly
on host in f64 (they need the exact initial transient and cost ~nothing).

v4 (fp8): panel + conv weights in float8_e4m3 with DoubleRow perf mode (2
contraction tiles per pass -> 2x PE throughput, ~half the DMA bytes).  The
conv weights ride at 32x scale so the small H-tap entries clear e4m3's
subnormal floor; the gram comes back 1024x hot and the host divides it out
(exact, power of two).  fp8 quantization of obs would bias yvar's diagonal,
but the packing error q = fp8(obs)-obs is known exactly on host, so kernel()
subtracts sum[q q^T + q obs^T + obs q^T] from Y (validated: rel err ~7e-4 vs
2e-2 budget).
"""
import numpy as np

T, N, B, J, W, NCORES = 100000, 32, 16, 8, 2720, 8
R = (T - W) // NCORES       # rows per core = 12160
NB = R // B                 # 760 blocks per core
PSI = NB + J                # 776 panel columns (incl halo)
NTS = 6                     # s-tiles per core
PS = [128, 128, 128, 128, 128, 120]
J0S = [0, 4]
SC = 32.0                   # fp8 weight scale (power of 2; host /1024 on gram)

# DMA descriptor-gen costs ~640ns per dma_start serialized per HWDGE ring and
# each descriptor another ~60ns on its queue, so each ring carries exactly ONE
# fat start into a pair-structured [128, 2, F] tile whose kc-chunks interleave
# everything the PE needs: pans | tri | gmat | halo.  The pair layout feeds
# fp8 DoubleRow matmuls (conv AND the gT chain).
#   bulka (sync ring):   chunk kc in {0,1}: pans(768) | triA(512) | gm(32) | halo(8)
#   bulkb (scalar ring): chunk kc in {2,3}: pans(768) | triB(288) | gm(32) | halo(8)
#   SWDGE: smalls (identb+cmn bf16), i32 (fp8), dstk (bf16)
FA = 768 + 512 + 32 + 8 + 8   # = 1328 cols per kc-chunk, ring A (pad: the
FB = 768 + 288 + 32 + 8 + 8   # = 1104, ring B; dual-fp8 LDWEIGHTS needs the
                              # k-tile stride 16-aligned)
A_COLS = 2 * FA
B_COLS = 2 * FB
S_COLS = 544                # smalls: identb(32) | cmn(512)

_PROG_CACHE = {}


def _build_device_consts(A64, C64, K64):
    import ml_dtypes
    bf16 = ml_dtypes.bfloat16
    fp8 = ml_dtypes.float8_e4m3
    M = A64 - K64 @ C64
    Mp = [np.eye(N)]
    for _ in range(B + 1):
        Mp.append(M @ Mp[-1])
    H = [C64 @ Mp[k] @ K64 for k in range(B)]
    TrilHneg = np.zeros((512, 512))
    for r in range(B):
        for t in range(r, B):
            TrilHneg[r*N:(r+1)*N, t*N:(t+1)*N] = -H[t - r].T
    # obs injection: obs[block, c] = panel[block, c+32] for steps 0..14
    for c in range(480):
        TrilHneg[c + 32, c] += 1.0
    # tri4[x, kc, c] = SC * TrilHneg[128kc + x, c], fp8
    tri4 = (SC * TrilHneg).reshape(4, 128, 512).transpose(1, 0, 2)
    tri4 = np.ascontiguousarray(tri4).astype(np.float32).astype(fp8)
    Gmat = np.zeros((512, N))
    for r in range(B):
        Gmat[r*N:(r+1)*N, :] = (Mp[B-1-r] @ K64).T
    gmat = np.ascontiguousarray(
        (SC * Gmat).reshape(4, 128, N).transpose(1, 0, 2).reshape(128, 128)
    ).astype(np.float32).astype(fp8)
    CMn = np.zeros((N, 512))
    for t in range(B):
        CMn[:, t*N:(t+1)*N] = -(C64 @ Mp[t+1]).T
    cmn = np.ascontiguousarray(CMn).astype(bf16)
    MB = Mp[B]
    D = [np.eye(N)]
    for _ in range(J - 1):
        D.append(MB @ D[-1])
    dstk = np.zeros((128, 128))
    for jg in range(J // 4):
        for rho in range(4):
            dstk[32*rho:32*rho+32, 32*jg:32*jg+32] = D[4*jg + rho].T
    dstk = dstk.astype(bf16)
    identb = np.eye(32).astype(bf16)
    i32 = (SC * np.eye(32)).astype(np.float32).astype(fp8)
    return tri4, gmat, cmn, dstk, identb, i32


def _host_exact(obs, A64, C64, K64, x0, Psqrt0):
    """f64 exact: P-series + outer(obs0) + eps outers for i < W."""
    obs64 = obs.astype(np.float64)
    M = A64 - K64 @ C64
    Y = np.outer(obs64[0], obs64[0])
    P = Psqrt0.astype(np.float64)
    for _ in range(4000):
        CP = C64 @ P
        Y += CP @ CP.T
        P = M @ P
        if np.abs(P).max() < 1e-16:
            break
    x = x0.astype(np.float64)
    for i in range(W):
        o_prev = obs64[i - 1] if i > 0 else obs64[T - 1]
        x = M @ x + K64 @ o_prev
        eps = obs64[i] - C64 @ x
        Y += np.outer(eps, eps)
    return Y


def _patch_tile_drain():
    """This walrus build allows only one sem wait per Drain; split the
    TileContext tail drain's waits across multiple drain instructions."""
    import concourse.tile as tile
    from concourse.vector_clock import ScopedClock
    if getattr(tile.TileContext, "_kf_drain_patched", False):
        return
    def _drain_and_barrier(self, tick_clock, wait_clock):
        nc = self.nc
        drain_inst = nc.sync.drain()
        wait_clock.add_sem_waits(drain_inst.ins, ScopedClock({None: tick_clock.global_clock}))
        si = drain_inst.ins.sync_info
        waits = list(si.on_wait or [])
        if len(waits) > 1:
            si.on_wait = waits[:1]
            for i in range(1, len(waits)):
                extra = nc.sync.drain()
                esi = extra.ins.sync_info
                if esi is None:
                    extra.ins.sync_info = type(si)(on_wait=waits[i:i+1], on_update=[])
                else:
                    esi.on_wait = waits[i:i+1]
        nc.all_engine_barrier(sem_only=True)
        assert self.sems is not None
        popped = nc._tile_sem_poison_stack.pop()
        assert popped is self._sem_poison
        nc.clear_and_free_semaphores(list(self.sems.allocated().values()))
    tile.TileContext._drain_and_barrier = _drain_and_barrier
    tile.TileContext._kf_drain_patched = True


def _split_multi_waits(nc):
    """This walrus build encodes at most one sem wait per instruction; hoist
    extra waits onto NoOps inserted just before in the same engine stream."""
    import concourse.mybir as mybir
    for func in nc.m.functions:
        for blk in func.blocks:
            insts = blk.instructions
            out, changed = [], False
            for inst in insts:
                si = inst.sync_info
                waits = list(si.on_wait) if si and si.on_wait else []
                if len(waits) > 1:
                    changed = True
                    for k, w in enumerate(waits[:-1]):
                        out.append(mybir.InstNoOp(
                            name=f"{inst.name}-hw{k}", engine=inst.engine,
                            bass_nofuse=True,
                            sync_info=mybir.SyncInfo(on_wait=[w], on_update=[])))
                    si.on_wait = [waits[-1]]
                out.append(inst)
            if changed:
                blk.instructions = out


def build_program(debug=False):
    import concourse.bass as bass
    import concourse.mybir as mybir
    import concourse.tile as tile
    _patch_tile_drain()
    f32 = mybir.dt.float32
    bf16 = mybir.dt.bfloat16
    fp8 = mybir.dt.float8e4
    DR = mybir.MatmulPerfMode.DoubleRow

    nc = bass.Bass()
    A_in = nc.declare_dram_parameter("bulka", [128, A_COLS], fp8, isOutput=False)
    B_in = nc.declare_dram_parameter("bulkb", [128, B_COLS], fp8, isOutput=False)
    S_in = nc.declare_dram_parameter("smalls", [32, S_COLS], bf16, isOutput=False)
    I_in = nc.declare_dram_parameter("i32", [32, 32], fp8, isOutput=False)
    D_in = nc.declare_dram_parameter("dstk", [128, 128], bf16, isOutput=False)
    yout = nc.declare_dram_parameter("yout", [128, 512], f32, isOutput=True)

    with tile.TileContext(nc) as tc:
        with (
            tc.tile_pool(name="big", bufs=1) as bpool,
            tc.tile_pool(name="work", bufs=1) as wpool,
            tc.tile_pool(name="etile", bufs=6) as epool,
            tc.tile_pool(name="ps2k", bufs=2, space="PSUM") as ppool,
            tc.tile_pool(name="epsum", bufs=5, space="PSUM") as eppool,
            tc.tile_pool(name="gramps", bufs=1, space="PSUM") as gpool,
        ):
            RAt = bpool.tile([128, 2, FA], fp8)
            RBt = bpool.tile([128, 2, FB], fp8)
            DSTK = bpool.tile([128, 128], bf16)
            S = wpool.tile([32, S_COLS], bf16)
            I32 = wpool.tile([32, 32], fp8)
            warm = wpool.tile([128, 128], bf16)
            zero512 = wpool.tile([128, 512], bf16)
            scx = wpool.tile([32, 32], bf16)

            # ---- one fat start per HWDGE ring
            nc.sync.dma_start(RAt[:, :, :], A_in[:, :])
            nc.scalar.dma_start(RBt[:, :, :], B_in[:, :])
            # scalar.copy lazily loads its ACT table (~1.3us) on first use;
            # trigger it now so the load overlaps the DMA window
            nc.scalar.copy(scx[:, 0:2], warm[0:32, 0:2])
            # ---- SWDGE: warm tiles first (they gate junk / prezero)
            nc.gpsimd.memset(warm[:], 0.0)
            nc.gpsimd.memset(zero512[:], 0.0)
            nc.gpsimd.dma_start(S[:], S_in[:])
            nc.gpsimd.dma_start(I32[:], I_in[:])
            nc.gpsimd.dma_start(DSTK[:], D_in[:])

            identb = S[:, 0:32]
            cmn = S[:, 32:544]

            gram_ps = gpool.tile([128, 512], f32)
            gramA = gram_ps[:, 0:256]
            gramB = gram_ps[:, 256:512]

            def junk(n):
                # PE keep-warm: HAM un-throttles only under sustained activity
                for _ in range(n):
                    nc.tensor.matmul(gram_ps[:, 0:128], lhsT=warm[:], rhs=warm[:],
                                     start=True, stop=True, skip_group_check=True)

            eps_list = [None] * NTS
            esb_list = [None] * NTS
            for st in range(NTS - 1):
                eps_list[st] = eppool.tile([128, 512], f32, tag="epsum",
                                           name=f"eps{st}")

            def prezero(sts):
                # pre-zero eps banks during the DMA wait so the conv
                # accumulations skip the start=True full-row-zero premium
                for st in sts:
                    nc.tensor.matmul(eps_list[st][:, :], lhsT=warm[:],
                                     rhs=zero512[:], start=True, stop=True,
                                     skip_group_check=True)

            junk(14)
            prezero([0, 1, 2, 3, 4])
            junk(2)

            # ---- gT [32, 776]: halo states + main panel states (split 512|264)
            # (DoubleRow LDWEIGHTS rejects 32-wide weights, so plain fp8 here)
            gtA = ppool.tile([32, 512], f32, tag="ps2k")
            gtB = ppool.tile([32, 248 + J], f32, tag="ps2k")
            def gh(kc):
                ring, tw = (RAt, 512) if kc < 2 else (RBt, 288)
                k, g0 = kc % 2, 768 + tw
                return (ring[:, k:k+1, g0:g0+32], ring[:, k:k+1, g0+32:g0+40],
                        ring[:, k:k+1, 0 : 512 - J], ring[:, k:k+1, 512 - J : 760])
            for kc in range(4):
                gm, halo, _, _ = gh(kc)
                nc.tensor.matmul(gtA[:, 0:J], lhsT=gm, rhs=halo,
                                 start=(kc == 0), stop=False)
            for kc in range(4):
                gm, _, pmain, ptail = gh(kc)
                nc.tensor.matmul(gtA[:, J : 512], lhsT=gm, rhs=pmain,
                                 start=False, stop=(kc == 3))
                nc.tensor.matmul(gtB[:, 0:248 + J], lhsT=gm, rhs=ptail,
                                 start=(kc == 0), stop=(kc == 3))

            def conv_st(st, start):
                p = PS[st]
                eps_ps = eps_list[st]
                nc.tensor.matmul(eps_ps[:p, 0:512],
                                 lhsT=RAt[:, :, 128*st : 128*st + p],
                                 rhs=RAt[:, :, 768:1280],
                                 start=(start and st == 5), stop=False,
                                 perf_mode=DR,
                                 skip_group_check=(st != 5))
                nc.tensor.matmul(eps_ps[:p, 224:512],
                                 lhsT=RBt[:, :, 128*st : 128*st + p],
                                 rhs=RBt[:, :, 768:1056],
                                 start=False, stop=False,
                                 perf_mode=DR,
                                 skip_group_check=(st != 5))

            def xcmn_st(st, start, xbt):
                p = PS[st]
                nc.tensor.matmul(eps_list[st][:p, :],
                                 lhsT=xbt[:, 128*st : 128*st+p],
                                 rhs=cmn[:, :],
                                 start=start, stop=(not start),
                                 skip_group_check=(st != 5))

            def obstail_st(st, stop):
                # obs step-15 term: panel position 0:32 of the NEXT block col
                p = PS[st]
                nc.tensor.matmul(eps_list[st][:p, 480:512],
                                 lhsT=RAt[0:32, 0:1, 128*st + 1 : 128*st + 1 + p],
                                 rhs=I32[:],
                                 start=False, stop=stop,
                                 skip_group_check=(st != 5))

            def stage_st(st, c0=0, c1=512):
                p = PS[st]
                if esb_list[st] is None:
                    esb_list[st] = epool.tile([128, 512], bf16, tag="etile",
                                              name=f"esb{st}")
                esb = esb_list[st]
                if c0 < 288:
                    nc.vector.tensor_copy(esb[:p, c0:288],
                                          eps_list[st][:p, c0:288])
                if c1 > 288:
                    nc.scalar.copy(esb[:p, 288:c1], eps_list[st][:p, 288:c1])

            def gram_st(st, first, gs=(0, 1, 2, 3)):
                p = PS[st]
                esb = esb_list[st]
                for g in gs:
                    # start=True zeroes the full 2KB bank row (both halves of
                    # the shared gram bank), so only the very first matmul of
                    # the whole gram accumulation may set it.
                    nc.tensor.matmul(gram_ps[:, 128*g : 128*g+128],
                                     lhsT=esb[:p, 128*g : 128*g+128],
                                     rhs=esb[:p, 128*g : 128*g+128],
                                     start=(first and g == 0),
                                     stop=(st == NTS - 1 and g in (1, 3)),
                                     skip_group_check=True)

            conv_st(0, start=True)
            obstail_st(0, stop=False)

            # ---- gts bf16 [32, 776]
            gts = wpool.tile([32, PSI], bf16)
            nc.vector.tensor_copy(gts[:, 0:512], gtA[:])
            nc.scalar.copy(gts[:, 512:PSI], gtB[:])



            # ---- gS [128, 776]: group rho = gT shifted right by rho cols
            gsA = ppool.tile([128, 512], f32, tag="ps2k")
            gsB = ppool.tile([128, 248 + J], f32, tag="ps2k")
            for rho in range(4):
                tp = (0, 32 * rho) if rho else None
                nc.tensor.matmul(gsA[32*rho : 32*rho+32, rho:512],
                                 lhsT=identb[:],
                                 rhs=gts[:, 0 : 512-rho],
                                 start=True, stop=True, tile_position=tp)
                nc.tensor.matmul(gsB[32*rho : 32*rho+32, 0:248 + J],
                                 lhsT=identb[:],
                                 rhs=gts[:, 512-rho : PSI-rho],
                                 start=True, stop=True, tile_position=tp)

            conv_st(1, start=True)
            obstail_st(1, stop=False)

            gss = wpool.tile([128, PSI], bf16)
            nc.vector.tensor_copy(gss[:, 0:512], gsA[:])
            nc.scalar.copy(gss[:, 512:PSI], gsB[:])

            conv_st(2, start=True)      # covers the gss-copy latency
            obstail_st(2, stop=False)

            # ---- XbT [32, 760]: sum_j D_j g_{s+15-j} via 2 tap-groups of 4
            xbtA = ppool.tile([32, 512], f32, tag="ps2k")
            xbtB = ppool.tile([32, 248], f32, tag="ps2k")
            for jg, j0 in enumerate(J0S):
                nc.tensor.matmul(xbtA[:, 0:512],
                                 lhsT=DSTK[:, 32*jg : 32*jg+32],
                                 rhs=gss[:, (J-1-j0) : (J-1-j0) + 512],
                                 start=(j0 == 0), stop=(j0 == J0S[-1]))
                nc.tensor.matmul(xbtB[:, 0:248],
                                 lhsT=DSTK[:, 32*jg : 32*jg+32],
                                 rhs=gss[:, (J-1-j0) + 512 : (J-1-j0) + 760],
                                 start=(j0 == 0), stop=(j0 == J0S[-1]))

            conv_st(3, start=True)
            obstail_st(3, stop=False)
            eps_list[5] = ppool.tile([128, 512], f32, tag="ps2k", name="eps5")

            xbt = wpool.tile([32, NB], bf16)
            nc.vector.tensor_copy(xbt[:, 0:512], xbtA[:])
            nc.scalar.copy(xbt[:, 512:NB], xbtB[:])

            # ---- sts 0-4 close with the xbt term so their convs are only
            # data-gated; st5 opens with it so its ADD->gram tail is short
            conv_st(4, start=True)
            obstail_st(4, stop=False)
            xcmn_st(0, start=False, xbt=xbt)
            stage_st(0)
            xcmn_st(1, start=False, xbt=xbt)
            stage_st(1)
            conv_st(5, start=True)
            obstail_st(5, stop=False)
            xcmn_st(2, start=False, xbt=xbt)
            stage_st(2)
            gram_st(0, first=True)
            xcmn_st(3, start=False, xbt=xbt)
            stage_st(3)
            gram_st(1, first=False)
            xcmn_st(4, start=False, xbt=xbt)
            stage_st(4)
            gram_st(2, first=False)
            xcmn_st(5, start=False, xbt=xbt)
            # last tile: stage/gram split by halves so each gram half closes
            # early and its copy+DMA overlaps the other half's tail
            ysb = wpool.tile([128, 512], f32)
            stage_st(5, 0, 288)
            stage_st(5, 288, 512)
            gram_st(3, first=False)
            gram_st(4, first=False)
            gram_st(5, first=False, gs=(0, 1))
            nc.scalar.copy(ysb[:, 0:256], gramA)
            nc.sync.dma_start(yout[:, 0:256], ysb[:, 0:256])
            gram_st(5, first=False, gs=(2, 3))
            nc.vector.tensor_copy(ysb[:, 256:512], gramB)
            nc.scalar.dma_start(yout[:, 256:512], ysb[:, 256:512])

    _split_multi_waits(nc)
    return nc


def _core_inputs(obs, c, consts):
    """Host-side layout prep for one core: pack bulka / bulkb / smalls."""
    import ml_dtypes
    bf16 = ml_dtypes.bfloat16
    fp8 = ml_dtypes.float8_e4m3
    tri4, gmat, cmn, dstk, identb, i32 = consts
    start = W + c * R
    hb = J * B + 1                                      # halo rows + 1
    flat = obs[start - hb : start + R]
    # panel rows (shifted by -1 obs row): s in [0, 760)
    pm = np.zeros((768, 512), np.float32)
    pm[:NB] = flat[hb - 1 : hb - 1 + R].reshape(NB, 512)
    # block col NB, position 0:32 feeds the step-15 obs term of block NB-1
    pm[NB, 0:32] = flat[hb - 1 + R]
    ptm = pm.reshape(768, 4, 128).transpose(2, 1, 0)    # [128, 4, 768]
    pth = np.zeros((128, 4, 32), np.float32)
    ph = flat[0 : J * B].reshape(J, 512)                # halo panel rows
    pth[:, :, :J] = ph.reshape(J, 4, 128).transpose(2, 1, 0)

    tri432 = tri4.astype(np.float32)
    gm32 = gmat.astype(np.float32)      # [128, 128]: cols 32kc+d
    bulka = np.zeros((128, A_COLS), np.float32)
    bulkb = np.zeros((128, B_COLS), np.float32)
    for kc in range(2):
        o = kc * FA
        bulka[:, o:o+768] = ptm[:, kc, :]
        bulka[:, o+768:o+1280] = tri432[:, kc, :]
        bulka[:, o+1280:o+1312] = gm32[:, 32*kc:32*kc+32]
        bulka[:, o+1312:o+1320] = pth[:, kc, 0:J]
    for kc in range(2, 4):
        o = (kc - 2) * FB
        bulkb[:, o:o+768] = ptm[:, kc, :]
        bulkb[:, o+768:o+1056] = tri432[:, kc, 224:512]
        bulkb[:, o+1056:o+1088] = gm32[:, 32*kc:32*kc+32]
        bulkb[:, o+1088:o+1096] = pth[:, kc, 0:J]

    smalls = np.zeros((32, S_COLS), np.float32)
    smalls[:, 0:32] = identb.astype(np.float32)
    smalls[:, 32:544] = cmn.astype(np.float32)

    return {"bulka": bulka.astype(fp8), "bulkb": bulkb.astype(fp8),
            "smalls": smalls.astype(bf16), "i32": i32,
            "dstk": dstk}


def kernel(observations, A, C, K, x0, Psqrt0, _trace=False, _trace_kwargs=None):
    import ml_dtypes
    fp8 = ml_dtypes.float8_e4m3
    obs = np.ascontiguousarray(observations, np.float32)
    A64 = np.asarray(A, np.float64)
    C64 = np.asarray(C, np.float64)
    K64 = np.asarray(K, np.float64)

    consts = _build_device_consts(A64, C64, K64)
    Y = _host_exact(obs, A64, C64, K64, np.asarray(x0), np.asarray(Psqrt0))

    # fp8 packing error of obs biases yvar through the o_cur identity path;
    # q is known exactly on host, so subtract its first+second order terms.
    ob = obs[W:].astype(np.float64)
    q = obs[W:].astype(fp8).astype(np.float64) - ob
    Y -= q.T @ q + q.T @ ob + ob.T @ q

    if "prog" not in _PROG_CACHE:
        _PROG_CACHE["prog"] = build_program()
    nc = _PROG_CACHE["prog"]

    in_maps = [_core_inputs(obs, c, consts) for c in range(NCORES)]

    from concourse.bass_utils import run_bass_kernel_spmd
    kw = dict(_trace_kwargs or {})
    res = run_bass_kernel_spmd(nc, in_maps, list(range(NCORES)), trace=_trace, **kw)

    for c in range(NCORES):
        G = np.asarray(res.results[c]["yout"], np.float64) / (SC * SC)
        for g in range(4):
            for tau in range(4):
                Y += G[32*tau:32*tau+32, 128*g+32*tau : 128*g+32*tau+32]
    loss = np.linalg.slogdet(Y / T)[1]
    out = np.float32(loss)
    if _trace:
        return out, res
    return out
